# revision 6
# baseline (speedup 1.0000x reference)
"""Grouped GEMM (MoE expert-parallel) on 8 TRN2 NeuronCores.

Expert-parallel: core e computes yT = W_e @ X_e^T; host transposes
back and rescales.  Strassen-Winograd + fp8 DoubleRow hybrid:

Winograd 7-product form with three fp8 DoubleRow quarter-K passes
folded into single-consumer product PSUM banks and DR22 staged via the
8th bank:
  M1=A11*B11  M2=A12*B21(+DR11)  M3=S4*B22(+DR12)  M4=A22*T4(-DR21)
  M5=S1*T1  M6=S2*T2  M7=S3*T3   (DR21 negated via host-shipped -x fp8)
  C11=M1+M2  U2=M1+M6  U3=U2+M7  U4=U2+M5  C12=U4+M3  C21=U3-M4
  C22=U3+M5+DR22

25 passes/position x 16 positions = 204800 row-cycles/GEMM (107.2us
ideal @1.91GHz) vs 229376 for the mixed kernel.  All four DR passes run
as ONE fp8 block per position (2 fp8<->bf16 PE mode transitions instead
of 8 -- worth ~1.5us/GEMM).  Combines pair-batch both token halves of
an f_ into [P,1024] bf16 staging (8 copies/pos split ACT/DVE, 8 DVE
tensor ops + 4 2KB-line y DMAs per pair).
"""

import numpy as np

import concourse.mybir as mybir
import concourse.tile as tile
from concourse import bacc

NUM_CORES = 8
IN_F = 1024
OUT_F = 2048
CAP = 2048
P = 128
KS = 768
KQ = 384
KO = 3
KF = 2
FB = OUT_F // P
FH = 8
TB = CAP // 512
TH = 2

BF16 = mybir.dt.bfloat16
FP8 = mybir.dt.float8e4
NP_BF16 = mybir.dt.np(BF16)
NP_FP8 = mybir.dt.np(FP8)
F32 = mybir.dt.float32
DR = mybir.MatmulPerfMode.DoubleRow
ADD = mybir.AluOpType.add
SUB = mybir.AluOpType.subtract
FP8_MAX = 240.0


def dedup_ldweights(nc):
    removed = 0
    for f in nc.m.functions:
        for bb in f.blocks:
            insts = bb.instructions
            last_sig = None
            victims = []
            for i in insts:
                if getattr(i, "engine", None) != mybir.EngineType.PE:
                    continue
                if isinstance(i, mybir.InstLdweights):
                    sig = (str(i.ins[0]), str(i.perf_mode),
                           str(i.is_transpose), str(i.tile_position))
                    if (sig == last_sig and not i.has_wait()
                            and not i.has_update()):
                        victims.append(i)
                    else:
                        last_sig = sig
                elif isinstance(i, mybir.InstMatmult):
                    pass
                else:
                    last_sig = None
            for v in victims:
                insts.remove(v)
            removed += len(victims)
    return removed


def _emit_gemm(nc, xfr, xfnr, wfq, aopr, bopq, ytr, pools):
    (x_pool, w_pool, a_pool, b_pool, t_pool, y_pool, psum_pool) = pools

    xf = x_pool.tile([P, KF, CAP], FP8, tag="xf", name="xf_res")
    xfn = x_pool.tile([P, KF, CAP], FP8, tag="xf", name="xfn_res")
    wf_tiles = [w_pool.tile([P, KF, P], FP8, tag="wf", name=f"wf_{fb}")
                for fb in range(FB)]
    aop_tiles = [a_pool.tile([P, KO, CAP // 2], BF16, tag="a",
                             name=f"a_{i}") for i in range(7)]
    bop_tiles = [[b_pool.tile([P, KO, P], BF16, tag="b", name=f"b_{i}_{f}")
                  for f in range(FH)] for i in range(7)]

    nc.sync.dma_start(xf[:], xfr[:])
    nc.sync.dma_start(xfn[:], xfnr[:])
    for fb in range(FB):
        nc.sync.dma_start(
            wf_tiles[fb][:], wfq[fb].rearrange("p (o f) -> p o f", o=KF))
    for i in range(7):
        nc.sync.dma_start(aop_tiles[i][:, :, 0:512],
                          aopr[i][:, :, 0:512])
    for i in range(7):
        nc.sync.dma_start(
            bop_tiles[i][0][:],
            bopq[i, 0].rearrange("p (o c) -> p o c", o=KO))
    for i in range(7):
        nc.sync.dma_start(aop_tiles[i][:, :, 512:1024],
                          aopr[i][:, :, 512:1024])
    for f in range(1, FH):
        for i in range(7):
            nc.sync.dma_start(
                bop_tiles[i][f][:],
                bopq[i, f].rearrange("p (o c) -> p o c", o=KO))

    # f_ outer / t_ inner: both token halves of an f_ are staged into
    # [P, 1024] buffers, then combined and written with half as many
    # tensor ops and 2KB-line y DMAs.
    for f_ in range(FH):
        mst = [t_pool.tile([P, TH * 512], BF16, tag="m", name=f"ms_{i}")
               for i in range(7)]
        d22 = t_pool.tile([P, TH * 512], BF16, tag="m", name="d22")
        for t_ in range(TH):
            ts0 = slice(t_ * 512, (t_ + 1) * 512)
            ts1 = slice((TH + t_) * 512, (TH + t_ + 1) * 512)
            hs = slice(t_ * 512, (t_ + 1) * 512)
            # All four DR passes in ONE fp8 block (2 fp8<->bf16 mode
            # transitions per position instead of 8), DR-first into the
            # product banks (accumulation order is free); identical wf
            # stationaries adjacent so dedup_ldweights keeps 2 of 4.
            ps_dr = psum_pool.tile([P, 512], F32, name="dr22", tag="psum")
            ms = [psum_pool.tile([P, 512], F32, name=f"m_{i}", tag="psum")
                  for i in range(7)]
            dr_first = {1, 2, 3}
            nc.tensor.matmul(ms[1], lhsT=wf_tiles[f_][:, :, :],
                             rhs=xf[:, :, ts0], start=True, stop=False,
                             perf_mode=DR)                # +DR11
            nc.tensor.matmul(ms[3], lhsT=wf_tiles[f_][:, :, :],
                             rhs=xfn[:, :, ts1], start=True, stop=False,
                             perf_mode=DR)                # -DR21
            nc.tensor.matmul(ms[2], lhsT=wf_tiles[FH + f_][:, :, :],
                             rhs=xf[:, :, ts0], start=True, stop=False,
                             perf_mode=DR)                # +DR12
            nc.tensor.matmul(ps_dr, lhsT=wf_tiles[FH + f_][:, :, :],
                             rhs=xf[:, :, ts1], start=True, stop=True,
                             perf_mode=DR)                # DR22 staged
            nc.scalar.copy(d22[:, hs], ps_dr[:])
            for i in range(7):
                for o in range(KO):
                    nc.tensor.matmul(
                        ms[i],
                        lhsT=bop_tiles[i][f_][:, o, :],
                        rhs=aop_tiles[i][:, o, ts0],
                        start=(o == 0 and i not in dr_first),
                        stop=(o == KO - 1),
                    )
            for i in (1, 3, 5):
                nc.scalar.copy(mst[i][:, hs], ms[i][:])
            for i in (0, 2, 4, 6):
                nc.vector.tensor_copy(mst[i][:, hs], ms[i][:])
        m1, m2, m3, m4, m5, m6, m7 = [m[:] for m in mst]
        u2 = t_pool.tile([P, TH * 512], BF16, tag="t", name="u2")
        u3 = t_pool.tile([P, TH * 512], BF16, tag="t", name="u3")
        u4 = t_pool.tile([P, TH * 512], BF16, tag="t", name="u4")
        t22 = t_pool.tile([P, TH * 512], BF16, tag="t", name="t22")
        y11 = y_pool.tile([P, TH * 512], BF16, tag="y", name="y11")
        y12 = y_pool.tile([P, TH * 512], BF16, tag="y", name="y12")
        y21 = y_pool.tile([P, TH * 512], BF16, tag="y", name="y21")
        y22 = y_pool.tile([P, TH * 512], BF16, tag="y", name="y22")
        nc.vector.tensor_tensor(y11[:], m1, m2, op=ADD)
        nc.vector.tensor_tensor(u2[:], m1, m6, op=ADD)
        nc.vector.tensor_tensor(u3[:], u2[:], m7, op=ADD)
        nc.vector.tensor_tensor(u4[:], u2[:], m5, op=ADD)
        nc.vector.tensor_tensor(y12[:], u4[:], m3, op=ADD)
        nc.vector.tensor_tensor(y21[:], u3[:], m4, op=SUB)
        nc.vector.tensor_tensor(t22[:], u3[:], m5, op=ADD)
        nc.vector.tensor_tensor(y22[:], t22[:], d22[:], op=ADD)
        nc.sync.dma_start(ytr[:, f_, 0:TH * 512], y11[:])
        nc.sync.dma_start(ytr[:, FH + f_, 0:TH * 512], y12[:])
        nc.sync.dma_start(ytr[:, f_, TH * 512:2 * TH * 512], y21[:])
        nc.sync.dma_start(ytr[:, FH + f_, TH * 512:2 * TH * 512], y22[:])


def _build(repeat: int = 1, hw_loop: int = 0):
    nc = bacc.Bacc(None, target_bir_lowering=False, debug=False)
    xft = nc.dram_tensor("xft", [KF * P, CAP], FP8, kind="ExternalInput")
    xfnt = nc.dram_tensor("xfnt", [KF * P, CAP], FP8, kind="ExternalInput")
    wfq = nc.dram_tensor("wfq", [FB, P, KF * P], FP8, kind="ExternalInput")
    aops = nc.dram_tensor("aops", [7, KQ, CAP // 2], BF16,
                          kind="ExternalInput")
    bops = nc.dram_tensor("bops", [7, FH, P, KQ], BF16,
                          kind="ExternalInput")
    yt = nc.dram_tensor("yt", [OUT_F, CAP], BF16, kind="ExternalOutput")
    xfr = xft.rearrange("(o p) m -> p o m", p=P)
    xfnr = xfnt.rearrange("(o p) m -> p o m", p=P)
    aopr = [aops[i].rearrange("(o p) m -> p o m", p=P) for i in range(7)]
    ytr = yt.rearrange("(fb p) m -> p fb m", p=P)

    with tile.TileContext(nc) as tc:
        with (
            tc.tile_pool(name="x_pool", bufs=4) as x_pool,
            tc.tile_pool(name="w_pool", bufs=FB + 2) as w_pool,
            tc.tile_pool(name="a_pool", bufs=12) as a_pool,
            tc.tile_pool(name="b_pool", bufs=7 * FH + 4) as b_pool,
            tc.tile_pool(name="t_pool", bufs=13) as t_pool,
            tc.tile_pool(name="y_pool", bufs=6) as y_pool,
            tc.tile_pool(name="psum", bufs=8, space="PSUM") as psum_pool,
        ):
            pools = (x_pool, w_pool, a_pool, b_pool, t_pool, y_pool,
                     psum_pool)
            if hw_loop:
                with tc.For_i(0, hw_loop):
                    for _ in range(repeat):
                        _emit_gemm(nc, xfr, xfnr, wfq, aopr, bops, ytr,
                                   pools)
            else:
                for _ in range(repeat):
                    _emit_gemm(nc, xfr, xfnr, wfq, aopr, bops, ytr, pools)
    dedup_ldweights(nc)
    nc.compile()
    return nc


_NC_CACHE: dict = {}


def _get_nc(repeat: int = 1, hw_loop: int = 0):
    key = (repeat, hw_loop)
    if key not in _NC_CACHE:
        _NC_CACHE[key] = _build(repeat, hw_loop)
    return _NC_CACHE[key]


def _pow2_scale(absmax: float) -> float:
    return float(2.0 ** np.ceil(np.log2(max(absmax, 1e-30) / FP8_MAX)))


def _pack_bop(bop):
    v = bop.reshape(KO, P, FH, P).transpose(2, 1, 0, 3)
    return np.ascontiguousarray(v.reshape(FH, P, KO * P))


def _pack_w(w_e):
    sw = _pow2_scale(np.abs(w_e).max())
    B = np.ascontiguousarray(w_e.T) / sw
    B11 = B[:KQ, :1024]
    B12 = B[:KQ, 1024:]
    B21 = B[KQ:KS, :1024]
    B22 = B[KQ:KS, 1024:]
    T1 = B12 - B11
    T2 = B22 - T1
    T3 = B22 - B12
    T4 = T2 - B21
    ops = [B11, B21, B22, T4, T1, T2, T3]
    bops = np.stack([_pack_bop(o) for o in ops]).astype(NP_BF16)
    wf = B[KS:].reshape(KF, P, FB, P).transpose(2, 1, 0, 3)
    wf = np.ascontiguousarray(wf.reshape(FB, P, KF * P))
    return bops, np.clip(wf, -FP8_MAX, FP8_MAX).astype(NP_FP8), sw


def _chunk_in_map(x, w_pack, off: int, size: int, sx: float):
    xe = np.zeros((CAP, IN_F), np.float32)
    if size > 0:
        xe[:size] = x[off:off + size]
    xs = xe / sx
    A11 = xs[:1024, :KQ]
    A12 = xs[:1024, KQ:KS]
    A21 = xs[1024:, :KQ]
    A22 = xs[1024:, KQ:KS]
    S1 = A21 + A22
    S2 = S1 - A11
    S3 = A11 - A21
    S4 = A12 - S2
    ops = [A11, A12, S4, A22, S1, S2, S3]
    aops = np.stack([np.ascontiguousarray(o.T) for o in ops])
    xq = np.clip(np.ascontiguousarray(xs[:, KS:].T), -FP8_MAX, FP8_MAX)
    return {
        "xft": xq.astype(NP_FP8),
        "xfnt": (-xq).astype(NP_FP8),
        "aops": aops.astype(NP_BF16),
        "bops": w_pack[0],
        "wfq": w_pack[1],
    }


_RUNNER_CACHE: dict = {}


def _get_runner():
    if "run" in _RUNNER_CACHE:
        return _RUNNER_CACHE["run"]

    import jax
    from jax.sharding import Mesh, PartitionSpec
    from jax.experimental.shard_map import shard_map
    from concourse import bass2jax
    from concourse.bass2jax import _bass_exec_p, install_neuronx_cc_hook

    nc = _get_nc(1)
    install_neuronx_cc_hook()
    assert nc.dbg_addr is None, "rebuild with debug=False"
    partition_name = (
        nc.partition_id_tensor.name if nc.partition_id_tensor else None
    )

    in_names, out_names, out_avals = [], [], []
    for alloc in nc.m.functions[0].allocations:
        if not isinstance(alloc, mybir.MemoryLocationSet):
            continue
        name = alloc.memorylocations[0].name
        if alloc.kind == "ExternalInput":
            if name != partition_name:
                in_names.append(name)
        elif alloc.kind == "ExternalOutput":
            out_names.append(name)
            out_avals.append(
                jax.core.ShapedArray(
                    tuple(alloc.tensor_shape), mybir.dt.np(alloc.dtype)
                )
            )
    n_params = len(in_names)
    all_in_names = list(in_names) + list(out_names)
    if partition_name is not None:
        all_in_names.append(partition_name)
    donate = tuple(range(n_params, n_params + len(out_names)))

    def _body(*args):
        operands = list(args)
        if partition_name is not None:
            operands.append(bass2jax.partition_id_tensor())
        outs = _bass_exec_p.bind(
            *operands,
            out_avals=tuple(out_avals),
            in_names=tuple(all_in_names),
            out_names=tuple(out_names),
            lowering_input_output_aliases=(),
            sim_require_finite=True,
            sim_require_nnan=True,
            nc=nc,
        )
        return tuple(outs)

    devices = jax.devices()[:NUM_CORES]
    mesh = Mesh(np.asarray(devices), ("core",))
    spec = PartitionSpec("core")
    fn = jax.jit(
        shard_map(
            _body, mesh=mesh,
            in_specs=(spec,) * (n_params + len(out_names)),
            out_specs=(spec,) * len(out_names),
            check_rep=False,
        ),
        donate_argnums=donate, keep_unused=True,
    )

    def run(in_maps):
        concat_in = [
            np.concatenate([np.asarray(m[k]) for m in in_maps], axis=0)
            for k in in_names
        ]
        zeros = [
            np.zeros((NUM_CORES * a.shape[0], *a.shape[1:]), a.dtype)
            for a in out_avals
        ]
        outs = fn(*concat_in, *zeros)
        arr = np.asarray(outs[0]).reshape(NUM_CORES, *out_avals[0].shape)
        return [{out_names[0]: arr[c]} for c in range(NUM_CORES)]

    _RUNNER_CACHE["run"] = run
    return run


def kernel(**inputs) -> np.ndarray:
    x = np.asarray(inputs["input_tokens"], dtype=np.float32)
    w = np.asarray(inputs["weight_stack"], dtype=np.float32)
    m_sizes = np.asarray(inputs["m_sizes"]).astype(np.int64)
    m_offsets = np.asarray(inputs["m_offsets"]).astype(np.int64)

    T = x.shape[0]
    E, O, K = w.shape
    assert K == IN_F and O == OUT_F and E == NUM_CORES

    sx = _pow2_scale(np.abs(x).max())
    w_packed = [_pack_w(w[e]) for e in range(E)]

    chunks = []
    for e in range(E):
        off, size = int(m_offsets[e]), int(m_sizes[e])
        off = max(0, min(off, T))
        size = max(0, min(size, T - off))
        pos = 0
        while pos < size:
            c = min(CAP, size - pos)
            chunks.append((e, off + pos, c))
            pos += c

    out = np.zeros((T, O), dtype=np.float32)
    run = _get_runner()
    for batch_start in range(0, len(chunks), NUM_CORES):
        batch = chunks[batch_start:batch_start + NUM_CORES]
        in_maps = [_chunk_in_map(x, w_packed[e], off, size, sx)
                   for (e, off, size) in batch]
        while len(in_maps) < NUM_CORES:
            in_maps.append(in_maps[0])
        results = run(in_maps)
        for i, (e, off, size) in enumerate(batch):
            yte = results[i]["yt"]
            scale = sx * w_packed[e][2]
            out[off:off + size] += (
                yte[:, :size].T.astype(np.float32) * scale)
    return out
